# revision 16
# baseline (speedup 1.0000x reference)
"""AttentionDecoder distributed Bass kernel for 8 TRN2 NeuronCores.

Model (per reference):
  emb = emb_table[input_tok]                       # [B, EMB]
  energy = tanh([h_rep, enc] @ attn_W + attn_b)    # [S, B, DEC]
  scores = einsum('sbd,d->bs', energy, v); a = softmax(scores, axis=1)
  weighted = einsum('bs,sbe->be', a, enc)          # [B, ENC]
  GRU step -> h_new                                # [B, DEC]
  pred = [h_new, weighted, emb] @ fc_W.T + fc_b    # [B, V]

Distribution: data-parallel over batch (32 rows/core) for attention+GRU,
AllGather of the transposed concat vector, vocab-parallel fc (4000 rows/core).

Host-side prep (layout only): shard tensors, transpose encoder_outputs to
[E, B, S], pre-transpose weights (fc_W/W_ih/W_hh -> [in, out]) and cast the
fc/GRU weights to bf16 (attention path stays f32; energy matmul runs f32r).
"""
import os
import sys
for p in ("/opt/trn_rl_repo", "/root/.axon_site/_ro/trn_rl_repo"):
    if p not in sys.path:
        sys.path.insert(0, p)

import numpy as np
import ml_dtypes

import concourse.bass as bass
import concourse.tile as tile
from concourse import bacc, mybir
from concourse.bass_utils import run_bass_kernel_spmd
from concourse.masks import make_identity

F32 = mybir.dt.float32
F32R = mybir.dt.float32r
BF16 = mybir.dt.bfloat16

B, S, ENC, DEC, EMB, V = 256, 128, 512, 512, 256, 32000
NC = 8
BL = B // NC          # 32 batch rows per core
VL = V // NC          # 4000 vocab rows per core
VLP = 4096            # padded vocab shard
KCAT = DEC + ENC + EMB  # 1280 = fc input dim
KGRU = EMB + ENC        # 768 = GRU input dim
AF = mybir.ActivationFunctionType
ALU = mybir.AluOpType


def _r(x):
    return np.ascontiguousarray(x)


def build_nc():
    nc = bacc.Bacc("TRN2", target_bir_lowering=False, debug=False, num_devices=NC)

    # ---- DRAM parameters (per-core shapes) ----
    encT_d = nc.dram_tensor("encT", [8, 128, 4, 4, S], F32R, kind="ExternalInput")
    h_d = nc.dram_tensor("h", [BL, DEC], F32, kind="ExternalInput")
    emb_d = nc.dram_tensor("emb", [BL, EMB], F32, kind="ExternalInput")
    attnWh_d = nc.dram_tensor("attnWh", [DEC, DEC], F32, kind="ExternalInput")
    attnWe_d = nc.dram_tensor("attnWe", [ENC, DEC], F32R, kind="ExternalInput")
    attnb_d = nc.dram_tensor("attnb", [128, 4], F32, kind="ExternalInput")
    v_d = nc.dram_tensor("vvec", [128, 4], F32R, kind="ExternalInput")
    wihT_d = nc.dram_tensor("wihT", [KGRU, 3 * DEC], BF16, kind="ExternalInput")
    whhT_d = nc.dram_tensor("whhT", [DEC, 3 * DEC], BF16, kind="ExternalInput")
    bsumrz_d = nc.dram_tensor("bsumrz", [1, 2 * DEC], F32, kind="ExternalInput")
    bihn_d = nc.dram_tensor("bihn", [1, DEC], F32, kind="ExternalInput")
    bhhn_d = nc.dram_tensor("bhhn", [1, DEC], F32, kind="ExternalInput")
    fcwT_d = nc.dram_tensor("fcwT", [16, 128, 10, 256], BF16, kind="ExternalInput")
    fcb_d = nc.dram_tensor("fcb", [1, VLP], BF16, kind="ExternalInput")

    pred_d = nc.dram_tensor("out_pred", [B, VLP], F32, kind="ExternalOutput")
    outh_d = nc.dram_tensor("out_h", [BL, DEC], F32, kind="ExternalOutput")

    with tile.TileContext(nc) as tc:
        with tc.tile_pool(name="consts", bufs=1) as consts, \
             tc.tile_pool(name="enc", bufs=1) as encp, \
             tc.tile_pool(name="wts", bufs=1) as wts, \
             tc.tile_pool(name="small", bufs=1) as small, \
             tc.tile_pool(name="tpool", bufs=2) as tpool, \
             tc.tile_pool(name="big", bufs=1) as bigp, \
             tc.tile_pool(name="fcw", bufs=4) as fcwp, \
             tc.tile_pool(name="outp", bufs=2) as outp, \
             tc.tile_pool(name="dram", bufs=1, space="DRAM") as dramp, \
             tc.tile_pool(name="psMM", bufs=2, space="PSUM") as psMM, \
             tc.tile_pool(name="psSC", bufs=2, space="PSUM") as psSC, \
             tc.tile_pool(name="psGRU", bufs=4, space="PSUM") as psGRU:

            # ================= loads =================
            ident = consts.tile([128, 128], F32)
            make_identity(nc, ident[:])

            attnWh_sb = consts.tile([128, 4, DEC], F32)
            nc.sync.dma_start(out=attnWh_sb[:],
                              in_=attnWh_d[:].rearrange("(kc p) d -> p kc d", p=128))
            attnWe_sb = consts.tile([128, 4, DEC], F32R)
            nc.sync.dma_start(out=attnWe_sb[:],
                              in_=attnWe_d[:].rearrange("(kc p) d -> p kc d", p=128))
            v_sb = consts.tile([128, 4], F32R)
            nc.sync.dma_start(out=v_sb[:], in_=v_d[:])

            encT_sb = encp.tile([128, 4, BL, S], F32R)   # [e_p, ec, b, s]
            for nt in range(8):
                nc.sync.dma_start(
                    out=encT_sb[:, :, 4 * nt:4 * nt + 4, :],
                    in_=encT_d[nt])

            h_sb = consts.tile([BL, DEC], F32)
            nc.sync.dma_start(out=h_sb[:], in_=h_d[:])
            emb_sb = consts.tile([BL, EMB], F32)
            nc.sync.dma_start(out=emb_sb[:], in_=emb_d[:])

            wihT_sb = wts.tile([128, 6, 3 * DEC], BF16)
            nc.sync.dma_start(out=wihT_sb[:],
                              in_=wihT_d[:].rearrange("(kc p) n -> p kc n", p=128))
            whhT_sb = wts.tile([128, 4, 3 * DEC], BF16)
            nc.sync.dma_start(out=whhT_sb[:],
                              in_=whhT_d[:].rearrange("(kc p) n -> p kc n", p=128))

            # biases broadcast across partitions (0-stride partition DMA from DRAM)
            def bcast_load(pool, dram_ap, parts, free, dtype=F32):
                t = pool.tile([parts, free], dtype)
                ap = bass.AP(dram_ap.tensor, dram_ap.offset,
                             [[0, parts]] + dram_ap.ap[1:])
                nc.sync.dma_start(out=t[:], in_=ap)
                return t

            bsumrz_sb = bcast_load(consts, bsumrz_d[:], BL, 2 * DEC)
            bihn_sb = bcast_load(consts, bihn_d[:], BL, DEC)
            bhhn_sb = bcast_load(consts, bhhn_d[:], BL, DEC)
            fcb_sb = None  # loaded later into the shared big slot

            # ================= hT (PE transpose of h) =================
            hT_sb = consts.tile([128, 4, BL], F32)
            hT_bf = consts.tile([128, 4, BL], BF16)
            for m in range(4):
                pst = psSC.tile([128, BL], F32, space="PSUM", tag="sc")
                nc.tensor.transpose(out=pst[:], in_=h_sb[:, m * 128:(m + 1) * 128],
                                    identity=ident[:BL, :BL])
                nc.vector.tensor_copy(out=hT_sb[:, m, :], in_=pst[:])
                nc.scalar.copy(out=hT_bf[:, m, :], in_=pst[:])

            # xcat_bf layout: [hnewT(0-3), wgtT(4-7), embT(8-9)] x [32 b]
            xcat_bf = consts.tile([128, 10, BL], BF16)
            for j in range(2):
                pst = psSC.tile([128, BL], F32, space="PSUM", tag="sc")
                nc.tensor.transpose(out=pst[:], in_=emb_sb[:, j * 128:(j + 1) * 128],
                                    identity=ident[:BL, :BL])
                nc.scalar.copy(out=xcat_bf[:, 8 + j, :], in_=pst[:])

            # ================= hWT[d, b] = (h @ W_h).T + attn_b =================
            attnb_sb = consts.tile([128, 4], F32)
            nc.sync.dma_start(out=attnb_sb[:], in_=attnb_d[:])
            hw_sb = consts.tile([128, 4, BL], F32)
            for m in range(4):
                psh = psSC.tile([128, BL], F32, space="PSUM", tag="sc")
                for kc in range(4):
                    nc.tensor.matmul(out=psh[:],
                                     lhsT=attnWh_sb[:, kc, m * 128:(m + 1) * 128],
                                     rhs=hT_sb[:, kc, :],
                                     start=(kc == 0), stop=(kc == 3))
                nc.scalar.activation(out=hw_sb[:, m, :], in_=psh[:],
                                     func=AF.Identity,
                                     bias=attnb_sb[:, m:m + 1], scale=1.0)

            # ============ energy + tanh + scores ============
            # E_T[d, (b, s)] tiles; (b,s) b-major, each ntile covers 4 b values
            scdram = dramp.tile([BL, S], F32)
            for nt in range(8):
                pssc = psSC.tile([1, 512], F32, space="PSUM", tag="sc")
                for m in range(4):
                    pse = psMM.tile([128, 4, S], F32, space="PSUM", tag="mm")
                    for kc in range(4):
                        nc.tensor.matmul(
                            out=pse[:],
                            lhsT=attnWe_sb[:, kc, m * 128:(m + 1) * 128],
                            rhs=encT_sb[:, kc, 4 * nt:4 * nt + 4, :],
                            start=(kc == 0), stop=(kc == 3))
                    hwslice = hw_sb[:, m, 4 * nt:4 * nt + 4]
                    hw_bc = bass.AP(hwslice.tensor, hwslice.offset,
                                    hwslice.ap + [[0, S]])
                    nc.vector.tensor_tensor(out=pse[:], in0=pse[:], in1=hw_bc,
                                            op=ALU.add)
                    t_tile = tpool.tile([128, 4, S], F32R)
                    nc.scalar.activation(out=t_tile[:], in_=pse[:], func=AF.Tanh)
                    nc.tensor.matmul(
                        out=pssc[:], lhsT=v_sb[:, m:m + 1],
                        rhs=t_tile[:].rearrange("p b s -> p (b s)"),
                        start=(m == 0), stop=(m == 3))
                scst = tpool.tile([1, 512], F32)
                nc.scalar.copy(out=scst[:], in_=pssc[:])
                nc.sync.dma_start(out=scdram[4 * nt:4 * nt + 4, :], in_=scst[:])

            # ============ scores -> [32, 128], softmax ============
            scores_bs = small.tile([BL, S], F32)
            nc.sync.dma_start(out=scores_bs[:], in_=scdram[:])
            negmax = small.tile([BL, 1], F32)
            nc.vector.tensor_reduce(out=negmax[:], in_=scores_bs[:],
                                    axis=mybir.AxisListType.X,
                                    op=ALU.max, negate=True)
            probs = small.tile([BL, S], F32)
            sumexp = small.tile([BL, 1], F32)
            nc.scalar.activation(out=probs[:], in_=scores_bs[:], func=AF.Exp,
                                 bias=negmax[:], scale=1.0, accum_out=sumexp[:])
            rsum = small.tile([BL, 1], F32)
            nc.vector.reciprocal(out=rsum[:], in_=sumexp[:])
            nc.vector.tensor_scalar_mul(probs[:], probs[:], rsum[:])

            # ====== weighted: wgtT[e, b] = sum_s a[b,s] * encT[e, b, s] ======
            aflat_dr = dramp.tile([BL, S], F32)
            nc.sync.dma_start(out=aflat_dr[:], in_=probs[:])
            a_rep = bigp.tile([128, BL, S], F32, tag="bigshare")
            af_ap = aflat_dr[:]
            nc.sync.dma_start(
                out=a_rep[:],
                in_=bass.AP(af_ap.tensor, af_ap.offset, [[0, 128]] + af_ap.ap))
            wgtT_sb = consts.tile([128, 4, BL], F32)
            for ec in range(4):
                for bq in range(4):
                    bsl = slice(8 * bq, 8 * bq + 8)
                    prod8 = bigp.tile([128, 8, S], F32, tag="prod8", bufs=2)
                    nc.gpsimd.tensor_tensor(out=prod8[:],
                                            in0=encT_sb[:, ec, bsl, :].bitcast(F32),
                                            in1=a_rep[:, bsl, :], op=ALU.mult)
                    nc.vector.tensor_reduce(out=wgtT_sb[:, ec, bsl],
                                            in_=prod8[:],
                                            axis=mybir.AxisListType.X, op=ALU.add)
            nc.scalar.copy(out=xcat_bf[:, 4:8, :], in_=wgtT_sb[:])

            # ============ GRU (per 512-slice: r, z, n) ============
            gru_src = [8, 9, 4, 5, 6, 7]  # emb chunks then weighted chunks
            # r and z: gi + gh accumulated in one PSUM group
            rz = small.tile([BL, 2 * DEC], F32)
            for sl in range(2):
                nsl = slice(512 * sl, 512 * (sl + 1))
                psg = psGRU.tile([BL, 512], F32, space="PSUM", tag="gru")
                for j, xs in enumerate(gru_src):
                    nc.tensor.matmul(out=psg[:], lhsT=xcat_bf[:, xs, :],
                                     rhs=wihT_sb[:, j, nsl],
                                     start=(j == 0), stop=False)
                for kc in range(4):
                    nc.tensor.matmul(out=psg[:], lhsT=hT_bf[:, kc, :],
                                     rhs=whhT_sb[:, kc, nsl],
                                     start=False, stop=(kc == 3))
                g = small.tile([BL, 512], F32, tag=f"g{sl}")
                nc.vector.tensor_tensor(out=g[:], in0=psg[:],
                                        in1=bsumrz_sb[:, nsl], op=ALU.add)
                nc.scalar.activation(out=rz[:, nsl], in_=g[:], func=AF.Sigmoid)
            # n = tanh(gi_n + b_ihn + r * (gh_n + b_hhn))
            nsl = slice(1024, 1536)
            psgi_n = psGRU.tile([BL, 512], F32, space="PSUM", tag="gru")
            for j, xs in enumerate(gru_src):
                nc.tensor.matmul(out=psgi_n[:], lhsT=xcat_bf[:, xs, :],
                                 rhs=wihT_sb[:, j, nsl],
                                 start=(j == 0), stop=(j == 5))
            psgh_n = psGRU.tile([BL, 512], F32, space="PSUM", tag="gru")
            for kc in range(4):
                nc.tensor.matmul(out=psgh_n[:], lhsT=hT_bf[:, kc, :],
                                 rhs=whhT_sb[:, kc, nsl],
                                 start=(kc == 0), stop=(kc == 3))
            hn = small.tile([BL, DEC], F32)
            nc.vector.tensor_tensor(out=hn[:], in0=psgh_n[:], in1=bhhn_sb[:],
                                    op=ALU.add)
            nin = small.tile([BL, DEC], F32)
            nc.vector.tensor_tensor(out=nin[:], in0=psgi_n[:], in1=bihn_sb[:],
                                    op=ALU.add)
            nc.vector.tensor_tensor(out=hn[:], in0=rz[:, 0:DEC], in1=hn[:],
                                    op=ALU.mult)
            nc.vector.tensor_tensor(out=nin[:], in0=nin[:], in1=hn[:], op=ALU.add)
            n_sb = small.tile([BL, DEC], F32)
            nc.scalar.activation(out=n_sb[:], in_=nin[:], func=AF.Tanh)
            # h_new = n + z * (h - n)
            d1 = small.tile([BL, DEC], F32)
            nc.vector.tensor_tensor(out=d1[:], in0=h_sb[:], in1=n_sb[:],
                                    op=ALU.subtract)
            nc.vector.tensor_tensor(out=d1[:], in0=rz[:, DEC:2 * DEC], in1=d1[:],
                                    op=ALU.mult)
            hnew = small.tile([BL, DEC], F32)
            nc.vector.tensor_tensor(out=hnew[:], in0=n_sb[:], in1=d1[:], op=ALU.add)
            nc.sync.dma_start(out=outh_d[:], in_=hnew[:])

            # hnewT into xcat slots 0-3
            for m in range(4):
                pst = psSC.tile([128, BL], F32, space="PSUM", tag="sc")
                nc.tensor.transpose(out=pst[:], in_=hnew[:, m * 128:(m + 1) * 128],
                                    identity=ident[:BL, :BL])
                nc.scalar.copy(out=xcat_bf[:, m, :], in_=pst[:])

            # ============ AllGather of xcat ============
            ag_in = dramp.tile([128, 10, BL], BF16)
            nc.sync.dma_start(out=ag_in[:], in_=xcat_bf[:])
            ag_out = dramp.tile([NC, 128, 10, BL], BF16, addr_space="Shared")
            nc.gpsimd.collective_compute(
                "AllGather", ALU.bypass,
                replica_groups=[list(range(NC))],
                ins=[ag_in[:].opt()], outs=[ag_out[:].opt()])

            # ============ fc: pred[b, vc] ============
            fcb_sb = bigp.tile([128, VLP], BF16, tag="bigshare")
            fcb_ap = fcb_d[:]
            nc.sync.dma_start(out=fcb_sb[:],
                              in_=bass.AP(fcb_ap.tensor, fcb_ap.offset,
                                          [[0, 128]] + fcb_ap.ap[1:]))
            xfull = []
            for bt in range(2):
                xf = wts.tile([128, 10, 128], BF16, name=f"xfull{bt}")
                nc.sync.dma_start(
                    out=xf[:].rearrange("p k (r b) -> p k r b", r=4),
                    in_=ag_out[4 * bt:4 * bt + 4, :, :, :]
                        .rearrange("r p k b -> p k r b"))
                xfull.append(xf)

            for q in range(16):
                fcw_sb = fcwp.tile([128, 10, 256], BF16)
                nc.sync.dma_start(out=fcw_sb[:], in_=fcwT_d[q])
                for bt in range(2):
                    psp = psMM.tile([128, 256], F32, space="PSUM", tag="mm")
                    for kc in range(10):
                        nc.tensor.matmul(out=psp[:], lhsT=xfull[bt][:, kc, :],
                                         rhs=fcw_sb[:, kc, :],
                                         start=(kc == 0), stop=(kc == 9))
                    osb = outp.tile([128, 256], F32)
                    vc0 = 256 * q
                    nc.vector.tensor_tensor(out=osb[:], in0=psp[:],
                                            in1=fcb_sb[:, vc0:vc0 + 256],
                                            op=ALU.add)
                    nc.sync.dma_start(
                        out=pred_d[128 * bt:128 * (bt + 1), vc0:vc0 + 256],
                        in_=osb[:])

    nc.compile()
    return nc


_NC_CACHE = {}


def kernel(input_tok, hidden, encoder_outputs, emb_table, attn_W, attn_b, v,
           W_ih, W_hh, b_ih, b_hh, fc_W, fc_b):
    input_tok = np.asarray(input_tok)
    hidden = np.asarray(hidden, dtype=np.float32)
    encoder_outputs = np.asarray(encoder_outputs, dtype=np.float32)
    emb_table = np.asarray(emb_table, dtype=np.float32)
    attn_W = np.asarray(attn_W, dtype=np.float32)
    attn_b = np.asarray(attn_b, dtype=np.float32)
    v = np.asarray(v, dtype=np.float32)
    W_ih = np.asarray(W_ih, dtype=np.float32)
    W_hh = np.asarray(W_hh, dtype=np.float32)
    b_ih = np.asarray(b_ih, dtype=np.float32)
    b_hh = np.asarray(b_hh, dtype=np.float32)
    fc_W = np.asarray(fc_W, dtype=np.float32)
    fc_b = np.asarray(fc_b, dtype=np.float32)

    # ---- host-side layout prep (sharding / transposes / dtype layout) ----
    encT = _r(encoder_outputs.transpose(2, 1, 0))          # [E, B, S]
    emb_rows = _r(emb_table[input_tok.astype(np.int64)])   # [B, EMB]
    wihT = _r(W_ih.T.astype(ml_dtypes.bfloat16))           # [768, 1536]
    whhT = _r(W_hh.T.astype(ml_dtypes.bfloat16))           # [512, 1536]
    attnb_r = _r(attn_b.reshape(4, 128).T)                 # [128, 4]
    v_r = _r(v.reshape(4, 128).T)                          # [128, 4]
    bmask = np.zeros((4, 512), dtype=np.float32)
    for j in range(4):
        bmask[j, j * 128:(j + 1) * 128] = 1.0
    bsumrz = _r((b_ih + b_hh)[None, 0:2 * DEC])
    bihn = _r(b_ih[None, 2 * DEC:3 * DEC])
    bhhn = _r(b_hh[None, 2 * DEC:3 * DEC])

    in_maps = []
    for c in range(NC):
        bs = slice(c * BL, (c + 1) * BL)
        vs = slice(c * VL, (c + 1) * VL)
        fcw_shard = np.zeros((VLP, KCAT), dtype=np.float32)
        fcw_shard[:VL] = fc_W[vs]
        fcb_shard = np.zeros((1, VLP), dtype=ml_dtypes.bfloat16)
        fcb_shard[0, :VL] = fc_b[vs].astype(ml_dtypes.bfloat16)
        in_maps.append({
            "encT": _r(encT[:, bs, :].reshape(4, 128, 8, 4, S)
                       .transpose(2, 1, 0, 3, 4)),
            "h": _r(hidden[0, bs, :]),
            "emb": _r(emb_rows[bs]),
            "attnWh": _r(attn_W[:DEC]),
            "attnWe": _r(attn_W[DEC:]),
            "attnb": attnb_r,
            "vvec": v_r,
            "wihT": wihT,
            "whhT": whhT,
            "bsumrz": bsumrz,
            "bihn": bihn,
            "bhhn": bhhn,
            "fcwT": _r(fcw_shard.T.astype(ml_dtypes.bfloat16)
                       .reshape(10, 128, 16, 256).transpose(2, 1, 0, 3)),
            "fcb": fcb_shard,
        })

    if "nc" not in _NC_CACHE:
        _NC_CACHE["nc"] = build_nc()
    nc = _NC_CACHE["nc"]

    trace = bool(int(os.environ.get("KERNEL_TRACE", "0")))
    kw = {}
    if trace:
        kw = dict(trace=True, tmpdir=os.environ.get("KERNEL_TRACE_DIR") or None)
    res = run_bass_kernel_spmd(nc, in_maps, core_ids=list(range(NC)), **kw)
    _NC_CACHE["last_results"] = res

    pred = np.concatenate([r["out_pred"][:, :VL] for r in res.results], axis=1)
    h_new = np.concatenate([r["out_h"] for r in res.results], axis=0)
    return pred.astype(np.float32), h_new.astype(np.float32)


# revision 17
# speedup vs baseline: 1.2308x; 1.2308x over previous
"""AttentionDecoder distributed Bass kernel for 8 TRN2 NeuronCores.

Model (per reference):
  emb = emb_table[input_tok]                       # [B, EMB]
  energy = tanh([h_rep, enc] @ attn_W + attn_b)    # [S, B, DEC]
  scores = einsum('sbd,d->bs', energy, v); a = softmax(scores, axis=1)
  weighted = einsum('bs,sbe->be', a, enc)          # [B, ENC]
  GRU step -> h_new                                # [B, DEC]
  pred = [h_new, weighted, emb] @ fc_W.T + fc_b    # [B, V]

Distribution: data-parallel over batch (32 rows/core) for attention+GRU,
AllGather of the transposed concat vector, vocab-parallel fc (4000 rows/core).

Host-side prep (layout only): shard tensors, transpose encoder_outputs to
[E, B, S], pre-transpose weights (fc_W/W_ih/W_hh -> [in, out]) and cast the
fc/GRU weights to bf16 (attention path stays f32; energy matmul runs f32r).
"""
import os
import sys
for p in ("/opt/trn_rl_repo", "/root/.axon_site/_ro/trn_rl_repo"):
    if p not in sys.path:
        sys.path.insert(0, p)

import numpy as np
import ml_dtypes

import concourse.bass as bass
import concourse.tile as tile
from concourse import bacc, mybir
from concourse.bass_utils import run_bass_kernel_spmd
from concourse.masks import make_identity

F32 = mybir.dt.float32
F32R = mybir.dt.float32r
BF16 = mybir.dt.bfloat16

B, S, ENC, DEC, EMB, V = 256, 128, 512, 512, 256, 32000
NC = 8
BL = B // NC          # 32 batch rows per core
VL = V // NC          # 4000 vocab rows per core
VLP = 4096            # padded vocab shard
KCAT = DEC + ENC + EMB  # 1280 = fc input dim
KGRU = EMB + ENC        # 768 = GRU input dim
AF = mybir.ActivationFunctionType
ALU = mybir.AluOpType


def _r(x):
    return np.ascontiguousarray(x)


def build_nc():
    nc = bacc.Bacc("TRN2", target_bir_lowering=False, debug=False, num_devices=NC)

    # ---- DRAM parameters (per-core shapes) ----
    encT_d = nc.dram_tensor("encT", [8, 128, 4, 4, S], F32R, kind="ExternalInput")
    h_d = nc.dram_tensor("h", [BL, DEC], F32, kind="ExternalInput")
    emb_d = nc.dram_tensor("emb", [BL, EMB], F32, kind="ExternalInput")
    attnWh_d = nc.dram_tensor("attnWh", [DEC, DEC], F32, kind="ExternalInput")
    attnWe_d = nc.dram_tensor("attnWe", [ENC, DEC], F32R, kind="ExternalInput")
    attnb_d = nc.dram_tensor("attnb", [128, 4], F32, kind="ExternalInput")
    v_d = nc.dram_tensor("vvec", [128, 4], F32R, kind="ExternalInput")
    wihT_d = nc.dram_tensor("wihT", [KGRU, 3 * DEC], BF16, kind="ExternalInput")
    whhT_d = nc.dram_tensor("whhT", [DEC, 3 * DEC], BF16, kind="ExternalInput")
    bsumrz_d = nc.dram_tensor("bsumrz", [1, 2 * DEC], F32, kind="ExternalInput")
    bihn_d = nc.dram_tensor("bihn", [1, DEC], F32, kind="ExternalInput")
    bhhn_d = nc.dram_tensor("bhhn", [1, DEC], F32, kind="ExternalInput")
    fcwT_d = nc.dram_tensor("fcwT", [16, 128, 10, 256], BF16, kind="ExternalInput")
    fcb_d = nc.dram_tensor("fcb", [1, VLP], BF16, kind="ExternalInput")

    pred_d = nc.dram_tensor("out_pred", [B, VLP], F32, kind="ExternalOutput")
    outh_d = nc.dram_tensor("out_h", [BL, DEC], F32, kind="ExternalOutput")

    with tile.TileContext(nc) as tc:
        with tc.tile_pool(name="consts", bufs=1) as consts, \
             tc.tile_pool(name="enc", bufs=1) as encp, \
             tc.tile_pool(name="wts", bufs=1) as wts, \
             tc.tile_pool(name="small", bufs=1) as small, \
             tc.tile_pool(name="tpool", bufs=2) as tpool, \
             tc.tile_pool(name="big", bufs=1) as bigp, \
             tc.tile_pool(name="fcw", bufs=4) as fcwp, \
             tc.tile_pool(name="outp", bufs=2) as outp, \
             tc.tile_pool(name="dram", bufs=1, space="DRAM") as dramp, \
             tc.tile_pool(name="psMM", bufs=2, space="PSUM") as psMM, \
             tc.tile_pool(name="psSC", bufs=2, space="PSUM") as psSC, \
             tc.tile_pool(name="psGRU", bufs=4, space="PSUM") as psGRU:

            # ================= loads =================
            ident = consts.tile([128, 128], F32)
            make_identity(nc, ident[:])

            attnWh_sb = consts.tile([128, 4, DEC], F32)
            nc.sync.dma_start(out=attnWh_sb[:],
                              in_=attnWh_d[:].rearrange("(kc p) d -> p kc d", p=128))
            attnWe_sb = consts.tile([128, 4, DEC], F32R)
            nc.sync.dma_start(out=attnWe_sb[:],
                              in_=attnWe_d[:].rearrange("(kc p) d -> p kc d", p=128))
            v_sb = consts.tile([128, 4], F32R)
            nc.sync.dma_start(out=v_sb[:], in_=v_d[:])

            encT_sb = encp.tile([128, 4, BL, S], F32R)   # [e_p, ec, b, s]
            for nt in range(8):
                nc.sync.dma_start(
                    out=encT_sb[:, :, 4 * nt:4 * nt + 4, :],
                    in_=encT_d[nt])

            h_sb = consts.tile([BL, DEC], F32)
            nc.sync.dma_start(out=h_sb[:], in_=h_d[:])
            emb_sb = consts.tile([BL, EMB], F32)
            nc.sync.dma_start(out=emb_sb[:], in_=emb_d[:])

            wihT_sb = wts.tile([128, 6, 3 * DEC], BF16)
            nc.sync.dma_start(out=wihT_sb[:],
                              in_=wihT_d[:].rearrange("(kc p) n -> p kc n", p=128))
            whhT_sb = wts.tile([128, 4, 3 * DEC], BF16)
            nc.sync.dma_start(out=whhT_sb[:],
                              in_=whhT_d[:].rearrange("(kc p) n -> p kc n", p=128))

            # biases broadcast across partitions (0-stride partition DMA from DRAM)
            def bcast_load(pool, dram_ap, parts, free, dtype=F32):
                t = pool.tile([parts, free], dtype)
                ap = bass.AP(dram_ap.tensor, dram_ap.offset,
                             [[0, parts]] + dram_ap.ap[1:])
                nc.sync.dma_start(out=t[:], in_=ap)
                return t

            bsumrz_sb = bcast_load(consts, bsumrz_d[:], BL, 2 * DEC)
            bihn_sb = bcast_load(consts, bihn_d[:], BL, DEC)
            bhhn_sb = bcast_load(consts, bhhn_d[:], BL, DEC)
            fcb_sb = None  # loaded later into the shared big slot

            # ================= hT (PE transpose of h) =================
            hT_sb = consts.tile([128, 4, BL], F32)
            hT_bf = consts.tile([128, 4, BL], BF16)
            for m in range(4):
                pst = psSC.tile([128, BL], F32, space="PSUM", tag="sc")
                nc.tensor.transpose(out=pst[:], in_=h_sb[:, m * 128:(m + 1) * 128],
                                    identity=ident[:BL, :BL])
                nc.vector.tensor_copy(out=hT_sb[:, m, :], in_=pst[:])
                nc.scalar.copy(out=hT_bf[:, m, :], in_=pst[:])

            # xcat_bf layout: [hnewT(0-3), wgtT(4-7), embT(8-9)] x [32 b]
            xcat_bf = consts.tile([128, 10, BL], BF16)
            for j in range(2):
                pst = psSC.tile([128, BL], F32, space="PSUM", tag="sc")
                nc.tensor.transpose(out=pst[:], in_=emb_sb[:, j * 128:(j + 1) * 128],
                                    identity=ident[:BL, :BL])
                nc.scalar.copy(out=xcat_bf[:, 8 + j, :], in_=pst[:])

            # ================= hWT[d, b] = (h @ W_h).T + attn_b =================
            attnb_sb = consts.tile([128, 4], F32)
            nc.sync.dma_start(out=attnb_sb[:], in_=attnb_d[:])
            hw_sb = consts.tile([128, 4, BL], F32)
            for m in range(4):
                psh = psSC.tile([128, BL], F32, space="PSUM", tag="sc")
                for kc in range(4):
                    nc.tensor.matmul(out=psh[:],
                                     lhsT=attnWh_sb[:, kc, m * 128:(m + 1) * 128],
                                     rhs=hT_sb[:, kc, :],
                                     start=(kc == 0), stop=(kc == 3))
                nc.scalar.activation(out=hw_sb[:, m, :], in_=psh[:],
                                     func=AF.Identity,
                                     bias=attnb_sb[:, m:m + 1], scale=1.0)

            # ============ energy + tanh + scores ============
            # E_T[d, (b, s)] tiles; (b,s) b-major, each ntile covers 4 b values
            scdram = dramp.tile([BL, S], F32)
            for nt in range(8):
                pssc = psSC.tile([1, 512], F32, space="PSUM", tag="sc")
                for m in range(4):
                    pse = psMM.tile([128, 4, S], F32, space="PSUM", tag="mm")
                    for kc in range(4):
                        nc.tensor.matmul(
                            out=pse[:],
                            lhsT=attnWe_sb[:, kc, m * 128:(m + 1) * 128],
                            rhs=encT_sb[:, kc, 4 * nt:4 * nt + 4, :],
                            start=(kc == 0), stop=(kc == 3))
                    hwslice = hw_sb[:, m, 4 * nt:4 * nt + 4]
                    hw_bc = bass.AP(hwslice.tensor, hwslice.offset,
                                    hwslice.ap + [[0, S]])
                    nc.vector.tensor_tensor(out=pse[:], in0=pse[:], in1=hw_bc,
                                            op=ALU.add)
                    t_tile = tpool.tile([128, 4, S], F32R)
                    nc.scalar.activation(out=t_tile[:], in_=pse[:], func=AF.Tanh)
                    nc.tensor.matmul(
                        out=pssc[:], lhsT=v_sb[:, m:m + 1],
                        rhs=t_tile[:].rearrange("p b s -> p (b s)"),
                        start=(m == 0), stop=(m == 3))
                scst = tpool.tile([1, 512], F32)
                nc.scalar.copy(out=scst[:], in_=pssc[:])
                nc.sync.dma_start(out=scdram[4 * nt:4 * nt + 4, :], in_=scst[:])

            # ============ scores -> [32, 128], softmax ============
            scores_bs = small.tile([BL, S], F32)
            nc.sync.dma_start(out=scores_bs[:], in_=scdram[:])
            negmax = small.tile([BL, 1], F32)
            nc.vector.tensor_reduce(out=negmax[:], in_=scores_bs[:],
                                    axis=mybir.AxisListType.X,
                                    op=ALU.max, negate=True)
            probs = small.tile([BL, S], F32)
            sumexp = small.tile([BL, 1], F32)
            nc.scalar.activation(out=probs[:], in_=scores_bs[:], func=AF.Exp,
                                 bias=negmax[:], scale=1.0, accum_out=sumexp[:])
            rsum = small.tile([BL, 1], F32)
            nc.vector.reciprocal(out=rsum[:], in_=sumexp[:])
            nc.vector.tensor_scalar_mul(probs[:], probs[:], rsum[:])

            # ====== weighted: wgtT[e, b] = sum_s a[b,s] * encT[e, b, s] ======
            aflat_dr = dramp.tile([BL, S], F32)
            nc.sync.dma_start(out=aflat_dr[:], in_=probs[:])
            a_rep = bigp.tile([128, BL, S], F32, tag="bigshare")
            af_ap = aflat_dr[:]
            nc.sync.dma_start(
                out=a_rep[:],
                in_=bass.AP(af_ap.tensor, af_ap.offset, [[0, 128]] + af_ap.ap))
            wgtT_sb = consts.tile([128, 4, BL], F32)
            for ec in range(4):
                for bq in range(4):
                    bsl = slice(8 * bq, 8 * bq + 8)
                    prod8 = bigp.tile([128, 8, S], F32, tag="prod8", bufs=2)
                    nc.vector.tensor_tensor(out=prod8[:],
                                            in0=encT_sb[:, ec, bsl, :].bitcast(F32),
                                            in1=a_rep[:, bsl, :], op=ALU.mult)
                    nc.vector.tensor_reduce(out=wgtT_sb[:, ec, bsl],
                                            in_=prod8[:],
                                            axis=mybir.AxisListType.X, op=ALU.add)
            nc.scalar.copy(out=xcat_bf[:, 4:8, :], in_=wgtT_sb[:])

            # ============ GRU (per 512-slice: r, z, n) ============
            gru_src = [8, 9, 4, 5, 6, 7]  # emb chunks then weighted chunks
            # r and z: gi + gh accumulated in one PSUM group
            rz = small.tile([BL, 2 * DEC], F32)
            for sl in range(2):
                nsl = slice(512 * sl, 512 * (sl + 1))
                psg = psGRU.tile([BL, 512], F32, space="PSUM", tag="gru")
                for j, xs in enumerate(gru_src):
                    nc.tensor.matmul(out=psg[:], lhsT=xcat_bf[:, xs, :],
                                     rhs=wihT_sb[:, j, nsl],
                                     start=(j == 0), stop=False)
                for kc in range(4):
                    nc.tensor.matmul(out=psg[:], lhsT=hT_bf[:, kc, :],
                                     rhs=whhT_sb[:, kc, nsl],
                                     start=False, stop=(kc == 3))
                g = small.tile([BL, 512], F32, tag=f"g{sl}")
                nc.vector.tensor_tensor(out=g[:], in0=psg[:],
                                        in1=bsumrz_sb[:, nsl], op=ALU.add)
                nc.scalar.activation(out=rz[:, nsl], in_=g[:], func=AF.Sigmoid)
            # n = tanh(gi_n + b_ihn + r * (gh_n + b_hhn))
            nsl = slice(1024, 1536)
            psgi_n = psGRU.tile([BL, 512], F32, space="PSUM", tag="gru")
            for j, xs in enumerate(gru_src):
                nc.tensor.matmul(out=psgi_n[:], lhsT=xcat_bf[:, xs, :],
                                 rhs=wihT_sb[:, j, nsl],
                                 start=(j == 0), stop=(j == 5))
            psgh_n = psGRU.tile([BL, 512], F32, space="PSUM", tag="gru")
            for kc in range(4):
                nc.tensor.matmul(out=psgh_n[:], lhsT=hT_bf[:, kc, :],
                                 rhs=whhT_sb[:, kc, nsl],
                                 start=(kc == 0), stop=(kc == 3))
            hn = small.tile([BL, DEC], F32)
            nc.vector.tensor_tensor(out=hn[:], in0=psgh_n[:], in1=bhhn_sb[:],
                                    op=ALU.add)
            nin = small.tile([BL, DEC], F32)
            nc.vector.tensor_tensor(out=nin[:], in0=psgi_n[:], in1=bihn_sb[:],
                                    op=ALU.add)
            nc.vector.tensor_tensor(out=hn[:], in0=rz[:, 0:DEC], in1=hn[:],
                                    op=ALU.mult)
            nc.vector.tensor_tensor(out=nin[:], in0=nin[:], in1=hn[:], op=ALU.add)
            n_sb = small.tile([BL, DEC], F32)
            nc.scalar.activation(out=n_sb[:], in_=nin[:], func=AF.Tanh)
            # h_new = n + z * (h - n)
            d1 = small.tile([BL, DEC], F32)
            nc.vector.tensor_tensor(out=d1[:], in0=h_sb[:], in1=n_sb[:],
                                    op=ALU.subtract)
            nc.vector.tensor_tensor(out=d1[:], in0=rz[:, DEC:2 * DEC], in1=d1[:],
                                    op=ALU.mult)
            hnew = small.tile([BL, DEC], F32)
            nc.vector.tensor_tensor(out=hnew[:], in0=n_sb[:], in1=d1[:], op=ALU.add)
            nc.sync.dma_start(out=outh_d[:], in_=hnew[:])

            # hnewT into xcat slots 0-3
            for m in range(4):
                pst = psSC.tile([128, BL], F32, space="PSUM", tag="sc")
                nc.tensor.transpose(out=pst[:], in_=hnew[:, m * 128:(m + 1) * 128],
                                    identity=ident[:BL, :BL])
                nc.scalar.copy(out=xcat_bf[:, m, :], in_=pst[:])

            # ============ AllGather of xcat ============
            ag_in = dramp.tile([128, 10, BL], BF16)
            nc.sync.dma_start(out=ag_in[:], in_=xcat_bf[:])
            ag_out = dramp.tile([NC, 128, 10, BL], BF16, addr_space="Shared")
            nc.gpsimd.collective_compute(
                "AllGather", ALU.bypass,
                replica_groups=[list(range(NC))],
                ins=[ag_in[:].opt()], outs=[ag_out[:].opt()])

            # ============ fc: pred[b, vc] ============
            fcb_sb = bigp.tile([128, VLP], BF16, tag="bigshare")
            fcb_ap = fcb_d[:]
            nc.sync.dma_start(out=fcb_sb[:],
                              in_=bass.AP(fcb_ap.tensor, fcb_ap.offset,
                                          [[0, 128]] + fcb_ap.ap[1:]))
            xfull = []
            for bt in range(2):
                xf = wts.tile([128, 10, 128], BF16, name=f"xfull{bt}")
                nc.sync.dma_start(
                    out=xf[:].rearrange("p k (r b) -> p k r b", r=4),
                    in_=ag_out[4 * bt:4 * bt + 4, :, :, :]
                        .rearrange("r p k b -> p k r b"))
                xfull.append(xf)

            for q in range(16):
                fcw_sb = fcwp.tile([128, 10, 256], BF16)
                nc.sync.dma_start(out=fcw_sb[:], in_=fcwT_d[q])
                for bt in range(2):
                    psp = psMM.tile([128, 256], F32, space="PSUM", tag="mm")
                    for kc in range(10):
                        nc.tensor.matmul(out=psp[:], lhsT=xfull[bt][:, kc, :],
                                         rhs=fcw_sb[:, kc, :],
                                         start=(kc == 0), stop=(kc == 9))
                    osb = outp.tile([128, 256], F32)
                    vc0 = 256 * q
                    nc.vector.tensor_tensor(out=osb[:], in0=psp[:],
                                            in1=fcb_sb[:, vc0:vc0 + 256],
                                            op=ALU.add)
                    nc.sync.dma_start(
                        out=pred_d[128 * bt:128 * (bt + 1), vc0:vc0 + 256],
                        in_=osb[:])

    nc.compile()
    return nc


_NC_CACHE = {}


def kernel(input_tok, hidden, encoder_outputs, emb_table, attn_W, attn_b, v,
           W_ih, W_hh, b_ih, b_hh, fc_W, fc_b):
    input_tok = np.asarray(input_tok)
    hidden = np.asarray(hidden, dtype=np.float32)
    encoder_outputs = np.asarray(encoder_outputs, dtype=np.float32)
    emb_table = np.asarray(emb_table, dtype=np.float32)
    attn_W = np.asarray(attn_W, dtype=np.float32)
    attn_b = np.asarray(attn_b, dtype=np.float32)
    v = np.asarray(v, dtype=np.float32)
    W_ih = np.asarray(W_ih, dtype=np.float32)
    W_hh = np.asarray(W_hh, dtype=np.float32)
    b_ih = np.asarray(b_ih, dtype=np.float32)
    b_hh = np.asarray(b_hh, dtype=np.float32)
    fc_W = np.asarray(fc_W, dtype=np.float32)
    fc_b = np.asarray(fc_b, dtype=np.float32)

    # ---- host-side layout prep (sharding / transposes / dtype layout) ----
    encT = _r(encoder_outputs.transpose(2, 1, 0))          # [E, B, S]
    emb_rows = _r(emb_table[input_tok.astype(np.int64)])   # [B, EMB]
    wihT = _r(W_ih.T.astype(ml_dtypes.bfloat16))           # [768, 1536]
    whhT = _r(W_hh.T.astype(ml_dtypes.bfloat16))           # [512, 1536]
    attnb_r = _r(attn_b.reshape(4, 128).T)                 # [128, 4]
    v_r = _r(v.reshape(4, 128).T)                          # [128, 4]
    bmask = np.zeros((4, 512), dtype=np.float32)
    for j in range(4):
        bmask[j, j * 128:(j + 1) * 128] = 1.0
    bsumrz = _r((b_ih + b_hh)[None, 0:2 * DEC])
    bihn = _r(b_ih[None, 2 * DEC:3 * DEC])
    bhhn = _r(b_hh[None, 2 * DEC:3 * DEC])

    in_maps = []
    for c in range(NC):
        bs = slice(c * BL, (c + 1) * BL)
        vs = slice(c * VL, (c + 1) * VL)
        fcw_shard = np.zeros((VLP, KCAT), dtype=np.float32)
        fcw_shard[:VL] = fc_W[vs]
        fcb_shard = np.zeros((1, VLP), dtype=ml_dtypes.bfloat16)
        fcb_shard[0, :VL] = fc_b[vs].astype(ml_dtypes.bfloat16)
        in_maps.append({
            "encT": _r(encT[:, bs, :].reshape(4, 128, 8, 4, S)
                       .transpose(2, 1, 0, 3, 4)),
            "h": _r(hidden[0, bs, :]),
            "emb": _r(emb_rows[bs]),
            "attnWh": _r(attn_W[:DEC]),
            "attnWe": _r(attn_W[DEC:]),
            "attnb": attnb_r,
            "vvec": v_r,
            "wihT": wihT,
            "whhT": whhT,
            "bsumrz": bsumrz,
            "bihn": bihn,
            "bhhn": bhhn,
            "fcwT": _r(fcw_shard.T.astype(ml_dtypes.bfloat16)
                       .reshape(10, 128, 16, 256).transpose(2, 1, 0, 3)),
            "fcb": fcb_shard,
        })

    if "nc" not in _NC_CACHE:
        _NC_CACHE["nc"] = build_nc()
    nc = _NC_CACHE["nc"]

    trace = bool(int(os.environ.get("KERNEL_TRACE", "0")))
    kw = {}
    if trace:
        kw = dict(trace=True, tmpdir=os.environ.get("KERNEL_TRACE_DIR") or None)
    res = run_bass_kernel_spmd(nc, in_maps, core_ids=list(range(NC)), **kw)
    _NC_CACHE["last_results"] = res

    pred = np.concatenate([r["out_pred"][:, :VL] for r in res.results], axis=1)
    h_new = np.concatenate([r["out_h"] for r in res.results], axis=0)
    return pred.astype(np.float32), h_new.astype(np.float32)
